# revision 3
# baseline (speedup 1.0000x reference)
"""Trainium2 Bass kernel for nn_BasePBC (PBC tap products).

Math:
  Reference computes, for each tap s=(m,n) with |m*n|<=25, |m|,|n|<=25:
      En  = roll(E, n); Emn = roll(E, m+n); Em = roll(E, m)   (roll along W)
      A   = En * conj(Emn);  Asum = A + flip_modes(A);  F = Asum * Em
  Key identities used here:
      roll(E,n)*conj(roll(E,m+n)) = roll(C_m, n) with C_m = E*conj(roll(E,m))
      Asum(mode0) == Asum(mode1) == roll(B_m, n),  B_m = sum_mu C_m[mu]
  So per tap:  F_mu[w] = B_m[w-n] * E_mu[w-m]   -- only 51 distinct B_m.

Distribution (SPMD, 8 cores, identical program):
  Shard W into 8 slices of 2048. Each core computes ALL 449 taps on its
  slice. Per-core differences live purely in the input data (a haloed
  window of E). On-chip layout puts (tap,b) rows on the 128 partitions.

Engine plan (v2):
  - B operand (shift n varies per row): indirect (gather) DMA from DRAM.
  - E operand (shift m, only 102 distinct (m,b) rows): replicated to the
    898 output rows by the TensorEngine with 0/1 selection matmuls from
    the gs tile (E shifted by m, already in SBUF for the B_m phase),
    PSUM -> SBUF fp16 copies on the Scalar(Act)/GpSimd(Pool) engines.
  - All DVE elementwise math uses scalar_tensor_tensor (4x_2p mode,
    2x faster than plain tensor_tensor).
"""

import numpy as np

import concourse.bass as bass
import concourse.bacc as bacc
import concourse.mybir as mybir
from concourse.tile import TileContext

# ---------------- problem constants (must match reference.py) --------------
RHO, L = 1.0, 50
TAPS = [
    (m, n)
    for m in range(-L // 2, L // 2 + 1)
    for n in range(-L // 2, L // 2 + 1)
    if abs(m * n) <= RHO * L // 2
]
S = len(TAPS)  # 449
B, W, NMODES = 2, 16384, 2
NCORES = 8
WLOC = W // NCORES  # 2048
EHALO = 64  # halo on each side of the local E window
EW = WLOC + 2 * EHALO  # 2176: e-plane row width
MS = sorted({m for m, _ in TAPS})  # -25..25
NM = len(MS)  # 51
M_IDX = {m: i for i, m in enumerate(MS)}
BMH = 32  # B_m halo (covers |n| <= 25)
BMW = WLOC + 2 * BMH  # 2112
NROWS = S * B  # 898   (row r = t*2 + b)
NB = 8
BR = 128  # rows per block; large DMAs must span all 128 partitions
#           (partial-partition stores run ~12x slower). 7 full blocks + one
#           2-row tail whose tiny store can afford the slow path.
NMB_PAD = 128  # B_m rows padded from 102 to 128 for the same reason
NMB_USED = NM * B  # 102 live (m,b) rows
NCOLS = 2 + NB  # offset-table columns
# Merged-gather column layouts (fp16 elements):
#   bm merged row:  Ar @ [0:2048],   Ai @ [2112:4160]           (run 4224)
#   bm-phase e row: 4 planes, 2112 wide each, starts 0/2176/4352/6528 (run 8640)
BRUN = 2 * BMW  # 4224
URUN = 3 * EW + BMW  # 8640

FP = mybir.dt.float16
NPFP = np.float16
ALU = mybir.AluOpType


def _pidx(b, mu, ri):
    return (b * 2 + mu) * 2 + ri


def _build_offsets() -> np.ndarray:
    offs = np.zeros((128, NCOLS), dtype=np.int32)
    # --- B_m phase (col 0: unshifted 4-plane run; col 1: shifted by m) ---
    for mi, m in enumerate(MS):
        for b in range(B):
            r = mi * 2 + b
            base = _pidx(b, 0, 0) * EW
            offs[r, 0] = base + (EHALO - BMH)
            offs[r, 1] = base + (EHALO - BMH) - m
    # --- F phase (col 2 + k: merged bm run for block k) ---
    for k in range(NB):
        r0 = k * BR
        for p in range(BR):
            r = r0 + p
            if r >= NROWS:
                break
            t, b = r // 2, r % 2
            m, n = TAPS[t]
            bmrow = M_IDX[m] * 2 + b
            offs[p, 2 + k] = (bmrow * 2) * BMW + BMH - n  # Ar..Ai run
    return offs


def _build_sel() -> np.ndarray:
    """0/1 selection matrices: sel[i, k*BR + j] = 1 iff gs source row i
    (= m_idx*2 + b) feeds output row k*BR + j.  lhsT of the replication
    matmuls (stationary: [K=102, 128])."""
    sel = np.zeros((128, NB * BR), dtype=NPFP)
    for r in range(NROWS):
        t, b = r // 2, r % 2
        m, _ = TAPS[t]
        sel[M_IDX[m] * 2 + b, r] = 1.0
    return sel


def _build_nc(reps: int = 1):
    nc = bacc.Bacc("TRN2", debug=False, target_bir_lowering=False)
    e_dram = nc.dram_tensor("e_planes", [8, EW], FP, kind="ExternalInput")
    offs_dram = nc.dram_tensor("offs", [128, NCOLS], mybir.dt.int32, kind="ExternalInput")
    sel_dram = nc.dram_tensor("sel", [128, NB * BR], FP, kind="ExternalInput")
    out_dram = nc.dram_tensor("out", [NROWS, 2, 2, WLOC], FP, kind="ExternalOutput")
    bm_dram = nc.dram_tensor("bm_scratch", [NMB_PAD, 2, BMW], FP)  # Internal scratch

    NMB = NMB_PAD  # padded to 128 partitions
    with TileContext(nc) as tc:
        with tc.tile_pool(name="const", bufs=1) as cpool:
            offs = cpool.tile([128, NCOLS], mybir.dt.int32)
            nc.sync.dma_start(out=offs[:], in_=offs_dram[:])
            sel = cpool.tile([128, NB * BR], FP)
            nc.sync.dma_start(out=sel[:], in_=sel_dram[:])
            for _rep in range(reps):
                _emit_body(nc, tc, offs, sel, e_dram, bm_dram, out_dram, NMB)
    nc.compile()
    return nc


def _emit_body(nc, tc, offs, sel, e_dram, bm_dram, out_dram, NMB):
    V = nc.vector

    def vmul(out, a, b):
        V.scalar_tensor_tensor(out=out, in0=a, scalar=1.0, in1=b, op0=ALU.mult, op1=ALU.mult)

    def vadd(out, a, b):
        V.scalar_tensor_tensor(out=out, in0=a, scalar=1.0, in1=b, op0=ALU.mult, op1=ALU.add)

    def vsub(out, a, b):  # out = a - b
        V.scalar_tensor_tensor(out=out, in0=b, scalar=-1.0, in1=a, op0=ALU.mult, op1=ALU.add)

    # gs (E shifted by m per (m,b) row) outlives the B_m phase: it is the
    # moving operand of the F-phase replication matmuls.
    with tc.tile_pool(name="gsp", bufs=1) as gspool:
        gs = gspool.tile([NMB, URUN], FP, tag="bmgs", name="bmgs")
        # ---------------- B_m phase ----------------
        with tc.tile_pool(name="bmph", bufs=1) as bpool:
            gu = bpool.tile([NMB, URUN], FP, tag="bmgu", name="bmgu")
            for t_, j in ((gu, 0), (gs, 1)):
                nc.gpsimd.indirect_dma_start(
                    out=t_[:],
                    out_offset=None,
                    in_=e_dram[:],
                    in_offset=bass.IndirectOffsetOnAxis(
                        ap=offs[:NMB, j : j + 1], axis=1
                    ),
                )
            ur0, ui0, ur1, ui1 = (gu[:, i * EW : i * EW + BMW] for i in range(4))
            sr0, si0, sr1, si1 = (gs[:, i * EW : i * EW + BMW] for i in range(4))
            bm = bpool.tile([NMB, 2, BMW], FP, tag="bm")
            tp = [bpool.tile([NMB, BMW], FP, tag=f"bmt{i}", name=f"bmt{i}") for i in range(4)]
            # real part: sum_mu (ur*sr + ui*si)
            vmul(tp[0][:], ur0, sr0)
            vmul(tp[1][:], ui0, si0)
            vmul(tp[2][:], ur1, sr1)
            vmul(tp[3][:], ui1, si1)
            vadd(tp[0][:], tp[0][:], tp[1][:])
            vadd(tp[2][:], tp[2][:], tp[3][:])
            vadd(bm[:, 0, :], tp[0][:], tp[2][:])
            # imag part: sum_mu (ui*sr - ur*si)
            vmul(tp[0][:], ui0, sr0)
            vmul(tp[1][:], ur0, si0)
            vmul(tp[2][:], ui1, sr1)
            vmul(tp[3][:], ur1, si1)
            vsub(tp[0][:], tp[0][:], tp[1][:])
            vsub(tp[2][:], tp[2][:], tp[3][:])
            vadd(bm[:, 1, :], tp[0][:], tp[2][:])
            nc.sync.dma_start(out=bm_dram[:], in_=bm[:])

        # ---------------- F phase ----------------
        with (
            tc.tile_pool(name="fop", bufs=2) as fpool,
            tc.tile_pool(name="ftmp", bufs=4) as tpool,
            tc.tile_pool(name="fout", bufs=3) as opool,
            tc.tile_pool(name="emsb", bufs=2) as empool,
            tc.tile_pool(name="emps", bufs=2, space="PSUM") as ppool,
        ):
            for k in range(NB):
                r0 = k * BR
                br = min(BR, NROWS - r0)  # last block: 2 rows (tiny
                # partial-partition store, ~32KB -- negligible)
                gbm = fpool.tile([128, BRUN], FP, tag="gbm", name="gbm")
                nc.gpsimd.indirect_dma_start(
                    out=gbm[:br],
                    out_offset=None,
                    in_=bm_dram[:],
                    in_offset=bass.IndirectOffsetOnAxis(
                        ap=offs[:br, 2 + k : 3 + k], axis=len(bm_dram.shape) - 1
                    ),
                )
                # Replicate E-shifted rows (gs) to this block's 128 output
                # rows: em plane order (er0, ei0, er1, ei1) <- gs planes
                # (mu0.re, mu0.im, mu1.re, mu1.im) at column BMH (shift m
                # cancels against the gather offset).
                em = empool.tile([128, 4, WLOC], FP, tag="em", name="em")
                for jj in range(4):
                    ps = ppool.tile([128, WLOC], mybir.dt.float32, tag="ps", name="ps")
                    for c in range(4):
                        nc.tensor.matmul(
                            out=ps[:, c * 512 : (c + 1) * 512],
                            lhsT=sel[:NMB_USED, r0 : r0 + BR],
                            rhs=gs[
                                :NMB_USED,
                                jj * EW + BMH + c * 512 : jj * EW + BMH + (c + 1) * 512,
                            ],
                        )
                    # GPSIMD cannot access PSUM (BIR verifier); Act does all
                    # four fp32->fp16 copies (~8.6us/block, under DMA budget).
                    nc.scalar.copy(out=em[:, jj, :], in_=ps[:])
                ar, ai = gbm[:br, 0:WLOC], gbm[:br, BMW : BMW + WLOC]
                f = opool.tile([128, 2, 2, WLOC], FP, tag="f")
                for mu in range(2):
                    er, ei = em[:br, 2 * mu, :], em[:br, 2 * mu + 1, :]
                    p = tpool.tile([128, WLOC], FP, tag="p", name="p")
                    q = tpool.tile([128, WLOC], FP, tag="q", name="q")
                    vmul(p[:br], ar, er)
                    vmul(q[:br], ai, ei)
                    vsub(f[:br, mu, 0, :], p[:br], q[:br])
                    p2 = tpool.tile([128, WLOC], FP, tag="p2", name="p2")
                    q2 = tpool.tile([128, WLOC], FP, tag="q2", name="q2")
                    vmul(p2[:br], ar, ei)
                    vmul(q2[:br], ai, er)
                    vadd(f[:br, mu, 1, :], p2[:br], q2[:br])
                nc.sync.dma_start(out=out_dram[r0 : r0 + br], in_=f[:br])


# ---------------- host side: cached compiled executable --------------------
_CACHE: dict = {}


def _get_runner(reps: int = 1):
    """Build nc once per reps and wrap a cached jitted SPMD executor
    (modeled on concourse.bass2jax.run_bass_via_pjrt, reusable across
    calls). reps>1 repeats the kernel body inside the NEFF (for timing)."""
    key = ("runner", reps)
    if key in _CACHE:
        return _CACHE[key]

    import jax
    from jax.sharding import Mesh, PartitionSpec
    from jax.experimental.shard_map import shard_map
    from concourse import bass2jax

    nc = _build_nc(reps)
    bass2jax.install_neuronx_cc_hook()

    partition_name = nc.partition_id_tensor.name if nc.partition_id_tensor else None
    in_names, out_names, out_avals = [], [], []
    for alloc in nc.m.functions[0].allocations:
        if not isinstance(alloc, mybir.MemoryLocationSet):
            continue
        name = alloc.memorylocations[0].name
        if alloc.kind == "ExternalInput":
            if name != partition_name:
                in_names.append(name)
        elif alloc.kind == "ExternalOutput":
            out_names.append(name)
            out_avals.append(
                jax.core.ShapedArray(tuple(alloc.tensor_shape), mybir.dt.np(alloc.dtype))
            )
    n_params = len(in_names)
    n_outs = len(out_avals)
    all_in_names = list(in_names) + list(out_names)
    if partition_name is not None:
        all_in_names.append(partition_name)
    donate = tuple(range(n_params, n_params + n_outs))

    def _body(*args):
        operands = list(args)
        if partition_name is not None:
            operands.append(bass2jax.partition_id_tensor())
        outs = bass2jax._bass_exec_p.bind(
            *operands,
            out_avals=tuple(out_avals),
            in_names=tuple(all_in_names),
            out_names=tuple(out_names),
            lowering_input_output_aliases=(),
            sim_require_finite=True,
            sim_require_nnan=True,
            nc=nc,
        )
        return tuple(outs)

    devices = jax.devices()[:NCORES]
    assert len(devices) == NCORES
    mesh = Mesh(np.asarray(devices), ("core",))
    in_specs = (PartitionSpec("core"),) * (n_params + n_outs)
    out_specs = (PartitionSpec("core"),) * n_outs
    smapped = shard_map(
        _body, mesh=mesh, in_specs=in_specs, out_specs=out_specs, check_rep=False
    )
    sharded = jax.jit(smapped, donate_argnums=donate, keep_unused=True)

    class Runner:
        pass

    R = Runner()
    R.sharded_nodonate = jax.jit(smapped, keep_unused=True)
    R.in_names, R.out_names, R.out_avals, R.mesh = in_names, out_names, out_avals, mesh

    def run(in_maps, device_only=False):
        concat_in = [
            np.concatenate([np.asarray(in_maps[c][nm]) for c in range(NCORES)], axis=0)
            for nm in in_names
        ]
        concat_zeros = [
            np.zeros((NCORES * av.shape[0], *av.shape[1:]), av.dtype) for av in out_avals
        ]
        out_arrs = sharded(*concat_in, *concat_zeros)
        if device_only:
            for o in out_arrs:
                o.block_until_ready()
            return None
        return [
            {
                nm: np.asarray(out_arrs[i]).reshape(NCORES, *out_avals[i].shape)[c]
                for i, nm in enumerate(out_names)
            }
            for c in range(NCORES)
        ]

    R.run = run
    _CACHE[key] = R
    return R


def _make_in_maps(E_real: np.ndarray, E_imag: np.ndarray):
    offs = _CACHE.get("offs")
    if offs is None:
        offs = _CACHE["offs"] = _build_offsets()
    sel = _CACHE.get("sel")
    if sel is None:
        sel = _CACHE["sel"] = _build_sel()
    E_real = np.asarray(E_real, dtype=np.float32)
    E_imag = np.asarray(E_imag, dtype=np.float32)
    in_maps = []
    for c in range(NCORES):
        idx = np.arange(c * WLOC - EHALO, (c + 1) * WLOC + EHALO) % W
        planes = np.empty((8, EW), dtype=NPFP)
        for b in range(B):
            for mu in range(NMODES):
                planes[_pidx(b, mu, 0)] = E_real[b, idx, mu].astype(NPFP)
                planes[_pidx(b, mu, 1)] = E_imag[b, idx, mu].astype(NPFP)
        in_maps.append({"e_planes": planes, "offs": offs, "sel": sel})
    return in_maps


def _assemble(results) -> np.ndarray:
    out = np.empty((B, W, NMODES, S), dtype=np.complex64)
    for c in range(NCORES):
        o = results[c]["out"][:NROWS].astype(np.float32).reshape(S, B, 2, 2, WLOC)
        cx = o[:, :, :, 0, :] + 1j * o[:, :, :, 1, :]  # [S, B, mu, WLOC]
        out[:, c * WLOC : (c + 1) * WLOC, :, :] = cx.transpose(1, 3, 2, 0)
    return out


def kernel(E_real: np.ndarray, E_imag: np.ndarray) -> np.ndarray:
    R = _get_runner()
    in_maps = _make_in_maps(E_real, E_imag)
    return _assemble(R.run(in_maps))


def _timed_loop(fn, args, n):
    import time
    import jax

    t0 = time.perf_counter()
    outs = [fn(*args) for _ in range(n)]
    jax.block_until_ready(outs)
    return time.perf_counter() - t0


def _device_args(R, E_real, E_imag):
    import jax
    from jax.sharding import NamedSharding, PartitionSpec

    in_maps = _make_in_maps(E_real, E_imag)
    concat_in = [
        np.concatenate([np.asarray(in_maps[c][nm]) for c in range(NCORES)], axis=0)
        for nm in R.in_names
    ]
    concat_zeros = [
        np.zeros((NCORES * av.shape[0], *av.shape[1:]), av.dtype) for av in R.out_avals
    ]
    shard = NamedSharding(R.mesh, PartitionSpec("core"))
    return [jax.device_put(a, shard) for a in (*concat_in, *concat_zeros)]


def bench(E_real: np.ndarray, E_imag: np.ndarray, iters: int = 40, hi_reps: int = 9):
    """Estimate on-device kernel time by differencing NEFFs with the body
    repeated 1x vs hi_reps inside a single execution (cancels per-call
    dispatch overhead through the tunnel). Returns (sec_per_kernel, None)."""
    import jax

    times = {}
    for reps in (1, hi_reps):
        R = _get_runner(reps)
        args = _device_args(R, E_real, E_imag)
        fn = R.sharded_nodonate
        jax.block_until_ready(fn(*args))  # compile+warm
        _timed_loop(fn, args, 3)
        best = min(_timed_loop(fn, args, iters) / iters for _ in range(3))
        times[reps] = best
        print(f"  reps={reps}: per-exec {best * 1e6:.0f} us")
    per_kernel = (times[hi_reps] - times[1]) / (hi_reps - 1)
    return per_kernel, None
